# revision 1
# baseline (speedup 1.0000x reference)
"""GCNConv (PyG-faithful, normalize=True, add_self_loops=True) on 8 Trainium2
NeuronCores via Bass/Tile.

Strategy (1D graph/data parallel):
  - Nodes are partitioned across the 8 cores (12500 rows each, padded to
    12544 = 98 blocks of 128).
  - Phase A: each core computes h_k = x_k @ W (fp32 matmuls), scales rows by
    dinv (symmetric GCN normalization, computed host-side from the edge
    index), casts to bf16 and AllGathers the scaled table
    g = dinv[:,None] * (x @ W) into every core's DRAM.
  - Phase B: each core owns 1/8 of the destination nodes. Edges (including
    self-loops) are host-sorted by destination block; per 128-edge tile a
    dma_gather (SWDGE, 4 queues round-robin) fetches g[src] rows (bf16), a
    host-precomputed one-hot selection tile (fp8, streamed from DRAM via
    HWDGE) feeds a TensorE matmul that segment-sums messages into a
    per-block PSUM accumulator. The epilogue scales by dinv_dst on ScalarE
    (PSUM->SBUF copy), adds bias per window on VectorE, and stores one
    window (896 nodes) per DMA.

  Per-(block, chunk) tile counts are computed from the actual edge data at
  call time (the program is compiled per call), maxed across cores so all 8
  cores run an identical (SPMD) program.
"""

import sys

if "/opt/trn_rl_repo" not in sys.path:
    sys.path.insert(0, "/opt/trn_rl_repo")

import numpy as np

P = 128          # partitions / tile edge count / feature dim
NCORES = 8
WBLK = 7         # blocks per window
CHUNKS = 4       # src chunks for int16 gather indices

_PAD_DL = 300    # sentinel dst_local for pad edges -> all-zero sel column


def _pack(x, edge_index, weight, b):
    """Host-side preprocessing: sharding, normalization metadata, gather
    index packing, one-hot sel tiles. All numpy, vectorized."""
    import ml_dtypes

    bias = b
    x = np.ascontiguousarray(np.asarray(x, dtype=np.float32))
    ei = np.asarray(edge_index)
    weight = np.ascontiguousarray(np.asarray(weight, dtype=np.float32))
    bias = np.asarray(bias, dtype=np.float32).reshape(-1)

    n, nin = x.shape
    nout = weight.shape[1]
    assert nin == P and nout == P, (nin, nout)
    assert n % NCORES == 0, n
    nb = n // NCORES                      # nodes per core (12500)
    blocks = (nb + P - 1) // P            # blocks per core (98)
    nbp = blocks * P                      # padded nodes per core (12544)
    npad = nbp * NCORES                   # padded table rows (100352)
    wblk = WBLK if blocks % WBLK == 0 else 1
    nwin = blocks // wblk                 # windows (14)
    chunk_rows = npad // CHUNKS           # rows per chunk (25088)
    assert chunk_rows < 32768, chunk_rows

    src = ei[0].astype(np.int64)
    dst = ei[1].astype(np.int64)

    deg = np.bincount(dst, minlength=n).astype(np.float32) + 1.0
    dinv = 1.0 / np.sqrt(deg)

    loop = np.arange(n, dtype=np.int64)
    src_a = np.concatenate([src, loop])
    dst_a = np.concatenate([dst, loop])
    m = src_a.shape[0]

    core = dst_a // nb
    dlc = dst_a - core * nb               # dst local to core
    blk = dlc >> 7
    dl = (dlc & 127).astype(np.int64)
    grow = (src_a // nb) * nbp + (src_a % nb)   # padded global row of src
    chunk = grow // chunk_rows
    rel = (grow % chunk_rows).astype(np.int16)

    key = (core * blocks + blk) * CHUNKS + chunk
    order = np.argsort(key, kind="stable")
    karr = core[order]
    relarr = rel[order]
    dlarr = dl[order]
    gkey = key[order]

    counts = np.bincount(key, minlength=NCORES * blocks * CHUNKS).reshape(
        NCORES, blocks, CHUNKS
    )
    t_bc = -(-counts.max(axis=0) // P)    # [blocks, CHUNKS] tiles per slot

    # global tile layout: for w in windows: for c in chunks: for b in window
    tile_off = np.zeros((blocks, CHUNKS), np.int64)
    wbase = np.zeros(nwin + 1, np.int64)
    col = 0
    for w in range(nwin):
        wbase[w] = col
        for c in range(CHUNKS):
            for bb in range(w * wblk, (w + 1) * wblk):
                tile_off[bb, c] = col
                col += t_bc[bb, c]
    t_total = int(col)
    wbase[nwin] = col

    # scatter edges into per-core packed arrays
    gs = np.zeros(NCORES * blocks * CHUNKS, np.int64)
    gs[1:] = np.cumsum(counts.ravel())[:-1]
    rank = np.arange(m, dtype=np.int64) - gs[gkey]
    base_flat = (tile_off * P).ravel()    # same for all cores
    dest = base_flat[(gkey % (blocks * CHUNKS))] + rank

    idx_lin = np.zeros((NCORES, t_total * P), np.int16)
    dl_lin = np.full((NCORES, t_total * P), _PAD_DL, np.int16)
    idx_lin[karr, dest] = relarr
    dl_lin[karr, dest] = dlarr.astype(np.int16)

    # wrap-16 + replicate to 128 partitions for dma_gather idx layout
    l16 = t_total * P // 16
    idx_w = idx_lin.reshape(NCORES, l16, 16).transpose(0, 2, 1)  # [8,16,L16]
    idx_pack = np.ascontiguousarray(np.tile(idx_w, (1, NCORES, 1)))  # [8,128,L16]

    # host-precomputed one-hot sel tiles, fp8: sel[e, gt, d] = (dl[gt,e] == d)
    sel_pack = np.empty((NCORES, P, t_total * P), ml_dtypes.float8_e4m3)
    dgrid = np.arange(P, dtype=np.int16)[None, None, :]
    for k in range(NCORES):
        dlr = dl_lin[k].reshape(t_total, P)          # [gt, e]
        sel_k = dlr.T[:, :, None] == dgrid           # [e, gt, d] bool
        sel_pack[k] = sel_k.reshape(P, t_total * P).astype(ml_dtypes.float8_e4m3)

    # per-core xT, dinv
    xt = np.zeros((NCORES, P, nbp), np.float32)
    dinv_t = np.zeros((NCORES, P, blocks), np.float32)
    for k in range(NCORES):
        xs = x[k * nb : (k + 1) * nb]
        xt[k, :, :nb] = xs.T
        dv = np.zeros(nbp, np.float32)
        dv[:nb] = dinv[k * nb : (k + 1) * nb]
        dinv_t[k] = dv.reshape(blocks, P).T
    bias_rep = np.ascontiguousarray(np.tile(bias[None, :], (P, 1)))

    meta = dict(
        n=n, nb=nb, blocks=blocks, nbp=nbp, npad=npad, nwin=nwin, wblk=wblk,
        chunk_rows=chunk_rows, t_bc=t_bc, tile_off=tile_off,
        wbase=wbase, t_total=t_total, l16=l16,
    )
    in_maps = [
        {
            "xt": xt[k],
            "w_in": weight,
            "bias": bias_rep,
            "dinv": dinv_t[k],
            "idxp": idx_pack[k],
            "selp": sel_pack[k],
        }
        for k in range(NCORES)
    ]
    return meta, in_maps


def _build_program(meta):
    from concourse import bass, bacc, mybir
    import concourse.tile as tile

    blocks = meta["blocks"]
    nbp = meta["nbp"]
    npad = meta["npad"]
    nwin = meta["nwin"]
    wblk = meta["wblk"]
    chunk_rows = meta["chunk_rows"]
    t_bc = meta["t_bc"]
    tile_off = meta["tile_off"]
    wbase = meta["wbase"]
    t_total = meta["t_total"]
    l16 = meta["l16"]
    jmax = int((wbase[1:] - wbase[:-1]).max())
    selmax = int(t_bc.sum(axis=1).max())  # max tiles per block

    f32 = mybir.dt.float32
    bf16 = mybir.dt.bfloat16
    fp8 = mybir.dt.float8e4

    nc = bacc.Bacc(num_swdge_queues=4)
    xt_in = nc.declare_dram_parameter("xt", [P, nbp], f32, isOutput=False)
    w_in = nc.declare_dram_parameter("w_in", [P, P], f32, isOutput=False)
    bias_in = nc.declare_dram_parameter("bias", [P, P], f32, isOutput=False)
    dinv_in = nc.declare_dram_parameter("dinv", [P, blocks], f32, isOutput=False)
    idx_in = nc.declare_dram_parameter("idxp", [P, l16], mybir.dt.int16, isOutput=False)
    sel_in = nc.declare_dram_parameter("selp", [P, t_total * P], fp8, isOutput=False)
    out_ext = nc.declare_dram_parameter("out", [nbp, P], f32, isOutput=True)

    h_shard = nc.dram_tensor("h_shard", [nbp, P], bf16)
    g_table = nc.dram_tensor("g_table", [npad, P], bf16, addr_space="Shared")

    with tile.TileContext(nc) as tc:
        with (
            tc.tile_pool(name="const", bufs=1) as cpool,
            tc.tile_pool(name="work", bufs=4) as wpool,
            tc.tile_pool(name="msgp", bufs=2) as mpool,
            tc.tile_pool(name="selp", bufs=6) as spool,
            tc.tile_pool(name="outp", bufs=2) as opool,
            tc.tile_pool(name="psA", bufs=2, space="PSUM") as psA,
            tc.tile_pool(name="psB", bufs=4, space="PSUM") as psB,
        ):
            # constants / metadata loads
            w_sb = cpool.tile([P, P], f32, tag="w")
            nc.sync.dma_start(out=w_sb[:], in_=w_in[:])
            bias_sb = cpool.tile([P, P], f32, tag="bias")
            nc.sync.dma_start(out=bias_sb[:], in_=bias_in[:])
            dinv_sb = cpool.tile([P, blocks], f32, tag="dinv")
            nc.sync.dma_start(out=dinv_sb[:], in_=dinv_in[:])
            idx_sb = cpool.tile([P, l16], mybir.dt.int16, tag="idx")
            for i in range(4):
                s = l16 // 4
                e = l16 if i == 3 else (i + 1) * s
                nc.sync.dma_start(out=idx_sb[:, i * s : e], in_=idx_in[:, i * s : e])

            # ---- phase A: h = x @ W, scale by dinv, cast bf16, allgather
            nchunk = next(d for d in (7, 8, 4, 2, 1) if blocks % d == 0)
            cw = nbp // nchunk            # nodes per chunk (1792 full-size)
            tpc = cw // P                 # tiles per chunk
            for ch in range(nchunk):
                xt_t = wpool.tile([P, cw], f32, tag="xt")
                nc.sync.dma_start(out=xt_t[:], in_=xt_in[:, ch * cw : (ch + 1) * cw])
                hbig = wpool.tile([P, tpc, P], bf16, tag="hbig")
                for t in range(tpc):
                    ph = psA.tile([P, P], f32, tag="ph")
                    nc.tensor.matmul(
                        out=ph[:],
                        lhsT=xt_t[:, t * P : (t + 1) * P],
                        rhs=w_sb[:],
                        start=True,
                        stop=True,
                    )
                    gb = ch * tpc + t
                    nc.vector.tensor_scalar(
                        out=hbig[:, t, :],
                        in0=ph[:],
                        scalar1=dinv_sb[:, gb : gb + 1],
                        scalar2=None,
                        op0=mybir.AluOpType.mult,
                    )
                nc.sync.dma_start(
                    out=h_shard[ch * cw : (ch + 1) * cw, :].rearrange(
                        "(t p) f -> p t f", p=P
                    ),
                    in_=hbig[:],
                )

            nc.gpsimd.collective_compute(
                "AllGather",
                mybir.AluOpType.bypass,
                replica_groups=[list(range(NCORES))],
                ins=[h_shard[:]],
                outs=[g_table[:]],
            )

            # ---- phase B: gather + one-hot segment matmul per dst block
            secmax = 0
            for w in range(nwin):
                for c in range(CHUNKS):
                    secmax = max(
                        secmax,
                        int(t_bc[w * wblk : (w + 1) * wblk, c].sum()),
                    )
            for w in range(nwin):
                msg = mpool.tile([P, jmax, P], bf16, tag="msg")
                sec0s = [0] * CHUNKS
                sels = [None] * CHUNKS
                for c in range(CHUNKS):
                    sec0 = None
                    seclen = 0
                    for bb in range(w * wblk, (w + 1) * wblk):
                        if t_bc[bb, c] > 0:
                            if sec0 is None:
                                sec0 = int(tile_off[bb, c])
                            seclen += int(t_bc[bb, c])
                    if seclen == 0:
                        continue
                    sec0s[c] = sec0
                    lo = sec0 - int(wbase[w])
                    nc.gpsimd.dma_gather(
                        out_ap=msg[:, lo : lo + seclen, :],
                        in_ap=g_table[c * chunk_rows : (c + 1) * chunk_rows, :],
                        idxs_ap=idx_sb[:, sec0 * 8 : (sec0 + seclen) * 8],
                        num_idxs=seclen * P,
                        num_idxs_reg=seclen * P,
                        elem_size=P,
                        single_packet=False,
                        queue_num=c,
                    )
                    # one sel DMA per section, on ScalarE's HWDGE queue
                    selw = spool.tile([P, secmax * P], fp8, tag="selw")
                    nc.scalar.dma_start(
                        out=selw[:, : seclen * P],
                        in_=sel_in[:, sec0 * P : (sec0 + seclen) * P],
                    )
                    sels[c] = selw
                osb_w = opool.tile([P, wblk, P], f32, tag="osbw")
                for j, bb in enumerate(range(w * wblk, (w + 1) * wblk)):
                    ntiles = int(t_bc[bb].sum())
                    assert ntiles > 0
                    acc = psB.tile([P, P], f32, tag="acc")
                    ti = 0
                    for c in range(CHUNKS):
                        tb = int(t_bc[bb, c])
                        for t in range(tb):
                            gt = int(tile_off[bb, c]) + t
                            mcol = gt - int(wbase[w])
                            st = gt - sec0s[c]
                            nc.tensor.matmul(
                                out=acc[:],
                                lhsT=sels[c][:, st * P : (st + 1) * P],
                                rhs=msg[:, mcol, :],
                                start=(ti == 0),
                                stop=(ti == ntiles - 1),
                            )
                            ti += 1
                    # epilogue: scale by dinv_dst on ScalarE (PSUM -> SBUF)
                    nc.scalar.activation(
                        out=osb_w[:, j, :],
                        in_=acc[:],
                        func=mybir.ActivationFunctionType.Copy,
                        scale=dinv_sb[:, bb : bb + 1],
                    )
                # bias add for the whole window on VectorE, then store
                nc.vector.tensor_tensor(
                    out=osb_w[:],
                    in0=osb_w[:],
                    in1=bias_sb[:].unsqueeze(1).to_broadcast([P, wblk, P]),
                    op=mybir.AluOpType.add,
                )
                nc.sync.dma_start(
                    out=out_ext[w * wblk * P : (w + 1) * wblk * P, :].rearrange(
                        "(j p) f -> p j f", p=P
                    ),
                    in_=osb_w[:],
                )

    nc.finalize()
    return nc


def _run(inputs, trace=False, trace_cores=None):
    from concourse.bass_utils import run_bass_kernel_spmd

    meta, in_maps = _pack(**inputs)
    nc = _build_program(meta)
    res = run_bass_kernel_spmd(
        nc,
        in_maps,
        list(range(NCORES)),
        trace=trace,
        trace_cores=trace_cores,
    )
    n, nb, nbp = meta["n"], meta["nb"], meta["nbp"]
    out = np.empty((n, P), np.float32)
    for k in range(NCORES):
        out[k * nb : (k + 1) * nb] = np.asarray(res.results[k]["out"])[:nb]
    return out, res


def kernel(x, edge_index, weight, b):
    out, _ = _run(dict(x=x, edge_index=edge_index, weight=weight, b=b))
    return out


if __name__ == "__main__":
    rng = np.random.default_rng(0)
    n, e = 100000, 1600000
    x = rng.standard_normal((n, P), dtype=np.float32)
    ei = rng.integers(0, n, (2, e)).astype(np.int64)
    w = (rng.standard_normal((P, P)) / np.sqrt(P)).astype(np.float32)
    bb = (rng.standard_normal(P) * 0.02).astype(np.float32)
    out = kernel(x, ei, w, bb)
    print("out", out.shape, out.dtype)



# revision 6
# speedup vs baseline: 1.5364x; 1.5364x over previous
"""GCNConv (PyG-faithful, normalize=True, add_self_loops=True) on 8 Trainium2
NeuronCores via Bass/Tile.

Strategy (1D graph/data parallel), v2:
  - Nodes are partitioned across the 8 cores (12500 rows each, padded to
    12544 = 98 blocks of 128).
  - Phase A: each core computes h_k = (dinv*x)_k @ W in bf16 (dinv folded
    host-side), writing the shard in four row-quarters.  After each quarter
    is stored, a quarter-wise AllGather replicates it into that quarter's
    chunk of the global message table (quarter-major layout), so phase B
    work on chunk c only waits on collective c.
  - Phase B: each core owns 1/8 of the destination nodes.  Edges (incl.
    self-loops) are host-sorted by (dst window of 2 blocks, table chunk,
    dst block); per (window, chunk) section one dma_gather (SWDGE, queue=c)
    fetches g[src] rows (bf16).  Padding inside a section is a tail of -1
    indices: the per-core true count is loaded at runtime (value_load ->
    num_idxs_reg) so pad descriptors are never generated or drained.
    Host-precomputed one-hot sel tiles (fp8) feed TensorE matmuls that
    segment-sum each block's messages into PSUM; tiles straddling the two
    blocks of a window are fed to both blocks' matmuls with separate sel.
    Epilogue scales by dinv_dst on ScalarE, adds bias on VectorE, stores.
  - Deep buffering (6 msg buffers over small 2-block windows) keeps the
    gather queues continuously fed instead of the 2-deep window pipeline.
"""

import sys

if "/opt/trn_rl_repo" not in sys.path:
    sys.path.insert(0, "/opt/trn_rl_repo")

import numpy as np

P = 128          # partitions / tile edge count / feature dim
NCORES = 8
WBLK = 2         # dst blocks per window
NCHUNK = 4       # table chunks == phase-A quarters
MBUFS = 6        # msg tile buffers (pipeline depth)
SBUFS = 6        # sel window buffers
OBUFS = 4


def _pack(x, edge_index, weight, b):
    """Host-side preprocessing: sharding, normalization, quarter-major table
    layout, section packing with -1 tail pads, one-hot sel tiles."""
    import ml_dtypes

    bf16 = ml_dtypes.bfloat16
    fp8 = ml_dtypes.float8_e4m3

    x = np.ascontiguousarray(np.asarray(x, dtype=np.float32))
    ei = np.asarray(edge_index)
    weight = np.ascontiguousarray(np.asarray(weight, dtype=np.float32))
    bias = np.asarray(b, dtype=np.float32).reshape(-1)

    n, nin = x.shape
    assert nin == P and weight.shape == (P, P)
    assert n % NCORES == 0
    nb = n // NCORES                      # nodes per core (12500)
    blocks = (nb + P - 1) // P            # blocks per core (98)
    nbp = blocks * P                      # padded nodes per core (12544)
    nw = blocks // WBLK                   # windows (49)
    assert blocks % WBLK == 0

    # quarters of the local shard (in blocks): 25,25,24,24
    qb = [blocks // NCHUNK + (1 if i < blocks % NCHUNK else 0) for i in range(NCHUNK)]
    qrows = [q * P for q in qb]                           # 3200,3200,3072,3072
    qstart = np.concatenate([[0], np.cumsum(qrows)])      # local row starts
    chunk_rows = [NCORES * r for r in qrows]              # table rows per chunk
    assert max(chunk_rows) < 32768

    src = ei[0].astype(np.int64)
    dst = ei[1].astype(np.int64)

    deg = np.bincount(dst, minlength=n).astype(np.float32) + 1.0
    dinv = 1.0 / np.sqrt(deg)

    loop = np.arange(n, dtype=np.int64)
    src_a = np.concatenate([src, loop])
    dst_a = np.concatenate([dst, loop])

    core = dst_a // nb
    dlc = dst_a - core * nb
    blk = dlc >> 7
    dl = (dlc & 127).astype(np.int16)
    win = blk // WBLK
    sub = (blk % WBLK).astype(np.int8)

    score = src_a // nb
    sloc = src_a % nb
    q = np.searchsorted(qstart[1:-1], sloc, side="right")  # quarter of src
    rel = (score * np.asarray(qrows)[q] + (sloc - qstart[q])).astype(np.int16)

    # one dummy edge per (core, win, chunk) so every section count >= 1
    dcore, dwin, dq = np.meshgrid(
        np.arange(NCORES), np.arange(nw), np.arange(NCHUNK), indexing="ij"
    )
    core = np.concatenate([core, dcore.ravel()])
    win = np.concatenate([win, dwin.ravel()])
    q = np.concatenate([q, dq.ravel()])
    sub = np.concatenate([sub, np.zeros(dcore.size, np.int8)])
    dl = np.concatenate([dl, np.full(dcore.size, -1, np.int16)])
    rel = np.concatenate([rel, np.zeros(dcore.size, np.int16)])

    key = (((core * nw + win) * NCHUNK + q) * 2 + sub).astype(np.int64)
    order = np.argsort(key, kind="stable")
    karr, warr, qarr, subarr = core[order], win[order], q[order], sub[order]
    dlarr, relarr = dl[order], rel[order]

    nsec = nw * NCHUNK
    gk = (karr * nsec + warr * NCHUNK + qarr).astype(np.int64)
    gk2 = gk * 2 + subarr
    cnt_sec = np.bincount(gk, minlength=NCORES * nsec).reshape(NCORES, nw, NCHUNK)
    cnt_sub = np.bincount(gk2, minlength=NCORES * nsec * 2).reshape(
        NCORES, nw, NCHUNK, 2
    )
    cnt0, cnt1 = cnt_sub[..., 0], cnt_sub[..., 1]

    cap = cnt_sec.max(axis=0)                        # [nw, NCHUNK]
    t_s = -(-cap // P)                               # static tiles per section

    # global static tile layout: window-major, chunks consecutive
    tile_base = np.zeros((nw, NCHUNK), np.int64)
    wbase = np.zeros(nw + 1, np.int64)
    col = 0
    for w in range(nw):
        wbase[w] = col
        for c in range(NCHUNK):
            tile_base[w, c] = col
            col += int(t_s[w, c])
    s_tiles = int(col)
    wbase[nw] = col
    jmax = int((wbase[1:] - wbase[:-1]).max())

    # per-core packed idx / dl / sub arrays over the static layout
    gs = np.zeros(NCORES * nsec, np.int64)
    gs[1:] = np.cumsum(np.bincount(gk, minlength=NCORES * nsec))[:-1]
    rank = np.arange(gk.size, dtype=np.int64) - gs[gk]
    base_flat = (tile_base * P).reshape(-1)          # same for all cores
    pos = base_flat[gk % nsec] + rank

    idx_lin = np.full((NCORES, s_tiles * P), -1, np.int16)
    dl_lin = np.full((NCORES, s_tiles * P), -1, np.int16)
    sub_lin = np.full((NCORES, s_tiles * P), -1, np.int8)
    idx_lin[karr, pos] = relarr
    dl_lin[karr, pos] = dlarr
    sub_lin[karr, pos] = subarr

    # static matmul tile ranges per (window, chunk, sub-block)
    # b=0: tiles [0, ceil(max cnt0 / P)); b=1: [min_cores(cnt0//P), t_s)
    hi0 = -(-cnt0.max(axis=0) // P)                  # [nw, NCHUNK]
    any1 = (cnt1 > 0).any(axis=0)
    c0m = np.where(cnt1 > 0, cnt0, np.iinfo(np.int64).max)
    lo1 = np.where(any1, c0m.min(axis=0) // P, 0)
    end1 = np.where(cnt1 > 0, cnt0 + cnt1, 0).max(axis=0)
    hi1 = np.where(any1, -(-end1 // P), 0)

    # sel entries: (w, c, t, b) in program order; per-window ranges
    sel_w, sel_c, sel_t, sel_b = [], [], [], []
    wsel_base = np.zeros(nw + 1, np.int64)
    mm_meta = []   # mm_meta[w][b] = list of (c, t, scol)
    for w in range(nw):
        wsel_base[w] = len(sel_w)
        mm_w = [[], []]
        for c in range(NCHUNK):
            for bb in range(2):
                lo = 0 if bb == 0 else int(lo1[w, c])
                hi = int(hi0[w, c]) if bb == 0 else int(hi1[w, c])
                for t in range(lo, hi):
                    scol = len(sel_w) - int(wsel_base[w])
                    mm_w[bb].append((c, t, scol))
                    sel_w.append(w)
                    sel_c.append(c)
                    sel_t.append(t)
                    sel_b.append(bb)
        mm_meta.append(mm_w)
    wsel_base[nw] = len(sel_w)
    s_sel = len(sel_w)
    selw_max = int((wsel_base[1:] - wsel_base[:-1]).max())
    sel_w = np.asarray(sel_w, np.int64)
    sel_c = np.asarray(sel_c, np.int64)
    sel_t = np.asarray(sel_t, np.int64)
    sel_b = np.asarray(sel_b, np.int8)

    # verify every real edge is covered by its block's static tile range
    t_of_pos = (pos - base_flat[gk % nsec]) // P
    lo_e = np.where(subarr == 0, 0, lo1[warr, qarr])
    hi_e = np.where(subarr == 0, hi0[warr, qarr], hi1[warr, qarr])
    assert (t_of_pos >= lo_e).all() and (t_of_pos < hi_e).all()

    # build sel tiles: sel[e, sid*P + d] = (sub==b & dl>=0 & dl==d)
    sel_gt = tile_base[sel_w, sel_c] + sel_t         # global tile per sel entry
    epos = (sel_gt[:, None] * P + np.arange(P)[None, :])  # [s_sel, P]
    dgrid = np.arange(P, dtype=np.int16)
    sel_pack = np.empty((NCORES, P, s_sel * P), fp8)
    for k in range(NCORES):
        dle = dl_lin[k][epos]                        # [s_sel, P]
        sbe = sub_lin[k][epos]
        m = (sbe == sel_b[:, None]) & (dle >= 0)
        oh = m[:, :, None] & (dle[:, :, None] == dgrid[None, None, :])
        # [s_sel, e, d] -> [e, s_sel, d]
        sel_pack[k] = (
            oh.transpose(1, 0, 2).reshape(P, s_sel * P).astype(fp8)
        )

    # wrap-16 + replicate to 128 partitions for dma_gather idx layout
    l16 = s_tiles * P // 16
    idx_w = idx_lin.reshape(NCORES, l16, 16).transpose(0, 2, 1)   # [8,16,L16]
    idx_pack = np.ascontiguousarray(np.tile(idx_w, (1, NCORES, 1)))  # [8,128,L16]

    cnts = np.ascontiguousarray(
        cnt_sec.reshape(NCORES, 1, nsec).astype(np.int32)
    )

    # per-core xT (dinv folded, bf16), dinv columns, bias
    xt = np.zeros((NCORES, P, nbp), bf16)
    dinv_t = np.zeros((NCORES, P, blocks), np.float32)
    for k in range(NCORES):
        xs = x[k * nb : (k + 1) * nb] * dinv[k * nb : (k + 1) * nb, None]
        xt[k, :, :nb] = xs.T.astype(bf16)
        dv = np.zeros(nbp, np.float32)
        dv[:nb] = dinv[k * nb : (k + 1) * nb]
        dinv_t[k] = dv.reshape(blocks, P).T
    w_bf = np.ascontiguousarray(weight.astype(bf16))
    bias_rep = np.ascontiguousarray(np.tile(bias[None, :], (P, 1)))

    meta = dict(
        n=n, nb=nb, blocks=blocks, nbp=nbp, nw=nw,
        qb=qb, qrows=qrows, chunk_rows=chunk_rows,
        t_s=t_s, tile_base=tile_base, wbase=wbase, s_tiles=s_tiles,
        jmax=jmax, l16=l16, s_sel=s_sel, selw_max=selw_max,
        wsel_base=wsel_base, mm_meta=mm_meta, nsec=nsec,
    )
    in_maps = [
        {
            "xt": xt[k],
            "w_in": w_bf,
            "bias": bias_rep,
            "dinv": dinv_t[k],
            "idxp": idx_pack[k],
            "selp": sel_pack[k],
            "cnts": cnts[k],
        }
        for k in range(NCORES)
    ]
    return meta, in_maps


def _build_program(meta):
    from concourse import bass, bacc, mybir
    import concourse.tile as tile

    blocks = meta["blocks"]
    nbp = meta["nbp"]
    nw = meta["nw"]
    qb = meta["qb"]
    qrows = meta["qrows"]
    chunk_rows = meta["chunk_rows"]
    t_s = meta["t_s"]
    tile_base = meta["tile_base"]
    wbase = meta["wbase"]
    s_tiles = meta["s_tiles"]
    jmax = meta["jmax"]
    l16 = meta["l16"]
    s_sel = meta["s_sel"]
    selw_max = meta["selw_max"]
    wsel_base = meta["wsel_base"]
    mm_meta = meta["mm_meta"]
    nsec = meta["nsec"]

    f32 = mybir.dt.float32
    bf16 = mybir.dt.bfloat16
    fp8 = mybir.dt.float8e4
    i16 = mybir.dt.int16
    i32 = mybir.dt.int32

    nc = bacc.Bacc(num_swdge_queues=4)
    xt_in = nc.declare_dram_parameter("xt", [P, nbp], bf16, isOutput=False)
    w_in = nc.declare_dram_parameter("w_in", [P, P], bf16, isOutput=False)
    bias_in = nc.declare_dram_parameter("bias", [P, P], f32, isOutput=False)
    dinv_in = nc.declare_dram_parameter("dinv", [P, blocks], f32, isOutput=False)
    idx_in = nc.declare_dram_parameter("idxp", [P, l16], i16, isOutput=False)
    sel_in = nc.declare_dram_parameter("selp", [P, s_sel * P], fp8, isOutput=False)
    cnt_in = nc.declare_dram_parameter("cnts", [1, nsec], i32, isOutput=False)
    out_ext = nc.declare_dram_parameter("out", [nbp, P], f32, isOutput=True)

    h_q = [nc.dram_tensor(f"h_q{c}", [qrows[c], P], bf16) for c in range(NCHUNK)]
    g_t = [
        nc.dram_tensor(f"g_t{c}", [chunk_rows[c], P], bf16, addr_space="Shared")
        for c in range(NCHUNK)
    ]

    with tile.TileContext(nc) as tc:
        with (
            tc.tile_pool(name="const", bufs=1) as cpool,
            tc.tile_pool(name="aph", bufs=2) as apool,
            tc.tile_pool(name="msgp", bufs=MBUFS) as mpool,
            tc.tile_pool(name="selp", bufs=SBUFS) as spool,
            tc.tile_pool(name="outp", bufs=OBUFS) as opool,
            tc.tile_pool(name="psA", bufs=2, space="PSUM") as psA,
            tc.tile_pool(name="psB", bufs=6, space="PSUM") as psB,
        ):
            # constants / metadata loads
            w_sb = cpool.tile([P, P], bf16, tag="w")
            nc.sync.dma_start(out=w_sb[:], in_=w_in[:])
            bias_sb = cpool.tile([P, P], f32, tag="bias")
            nc.sync.dma_start(out=bias_sb[:], in_=bias_in[:])
            dinv_sb = cpool.tile([P, blocks], f32, tag="dinv")
            nc.sync.dma_start(out=dinv_sb[:], in_=dinv_in[:])
            cnt_sb = cpool.tile([1, nsec], i32, tag="cnt")
            nc.sync.dma_start(out=cnt_sb[:], in_=cnt_in[:])
            idx_sb = cpool.tile([P, l16], i16, tag="idx")
            for i in range(4):
                s = l16 // 4
                e = l16 if i == 3 else (i + 1) * s
                nc.sync.dma_start(out=idx_sb[:, i * s : e], in_=idx_in[:, i * s : e])

            # prime msg buffers to zero so never-gathered tails are finite
            # (sel is zero there; avoids NaN*0 from uninitialized SBUF)
            for _ in range(MBUFS):
                mz = mpool.tile([P, jmax, P], bf16, tag="msg")
                nc.vector.memset(mz[:], 0)

            # ---- phase A: h = (dinv*x) @ W per quarter, then quarter AllGather
            qs = 0
            for c in range(NCHUNK):
                rows = qrows[c]
                xa = apool.tile([P, rows], bf16, tag="xa")
                nc.sync.dma_start(out=xa[:], in_=xt_in[:, qs : qs + rows])
                hq = apool.tile([P, qb[c], P], bf16, tag="hq")
                for t in range(qb[c]):
                    ph = psA.tile([P, P], f32, tag="ph")
                    nc.tensor.matmul(
                        out=ph[:],
                        lhsT=xa[:, t * P : (t + 1) * P],
                        rhs=w_sb[:],
                        start=True,
                        stop=True,
                    )
                    nc.scalar.activation(
                        out=hq[:, t, :],
                        in_=ph[:],
                        func=mybir.ActivationFunctionType.Copy,
                    )
                nc.sync.dma_start(
                    out=h_q[c][:].rearrange("(t p) f -> p t f", p=P),
                    in_=hq[:],
                )
                nc.gpsimd.collective_compute(
                    "AllGather",
                    mybir.AluOpType.bypass,
                    replica_groups=[list(range(NCORES))],
                    ins=[h_q[c][:]],
                    outs=[g_t[c][:]],
                )
                qs += rows

            # ---- phase B: gather + one-hot segment matmul per window
            cnt_regs = [nc.gpsimd.alloc_register(f"cntq{i}") for i in range(8)]
            for w in range(nw):
                msg = mpool.tile([P, jmax, P], bf16, tag="msg")
                nsel = int(wsel_base[w + 1] - wsel_base[w])
                selw = spool.tile([P, selw_max * P], fp8, tag="selw")
                seng = nc.scalar if (w % 2 == 0) else nc.sync
                seng.dma_start(
                    out=selw[:, : nsel * P],
                    in_=sel_in[:, int(wsel_base[w]) * P : (int(wsel_base[w]) + nsel) * P],
                )
                for c in range(NCHUNK):
                    tc_ = int(t_s[w, c])
                    if tc_ == 0:
                        continue
                    sec0 = int(tile_base[w, c])
                    lo = sec0 - int(wbase[w])
                    reg = cnt_regs[c + 4 * (w % 2)]
                    nc.gpsimd.reg_load(
                        reg, cnt_sb[0:1, w * NCHUNK + c : w * NCHUNK + c + 1]
                    )
                    nc.gpsimd.dma_gather(
                        out_ap=msg[:, lo : lo + tc_, :],
                        in_ap=g_t[c][:],
                        idxs_ap=idx_sb[:, sec0 * 8 : (sec0 + tc_) * 8],
                        num_idxs=tc_ * P,
                        num_idxs_reg=reg,
                        elem_size=P,
                        single_packet=False,
                        queue_num=c,
                    )
                osb = opool.tile([P, WBLK, P], f32, tag="osb")
                for bb in range(WBLK):
                    mml = mm_meta[w][bb]
                    assert mml
                    acc = psB.tile([P, P], f32, tag="acc")
                    for i, (c, t, scol) in enumerate(mml):
                        mcol = int(tile_base[w, c]) + t - int(wbase[w])
                        nc.tensor.matmul(
                            out=acc[:],
                            lhsT=selw[:, scol * P : (scol + 1) * P],
                            rhs=msg[:, mcol, :],
                            start=(i == 0),
                            stop=(i == len(mml) - 1),
                        )
                    gb = w * WBLK + bb
                    nc.scalar.activation(
                        out=osb[:, bb, :],
                        in_=acc[:],
                        func=mybir.ActivationFunctionType.Copy,
                        scale=dinv_sb[:, gb : gb + 1],
                    )
                nc.vector.tensor_tensor(
                    out=osb[:],
                    in0=osb[:],
                    in1=bias_sb[:].unsqueeze(1).to_broadcast([P, WBLK, P]),
                    op=mybir.AluOpType.add,
                )
                nc.sync.dma_start(
                    out=out_ext[w * WBLK * P : (w + 1) * WBLK * P, :].rearrange(
                        "(j p) f -> p j f", p=P
                    ),
                    in_=osb[:],
                )

    nc.finalize()
    return nc


def _run(inputs, trace=False, trace_cores=None):
    from concourse.bass_utils import run_bass_kernel_spmd

    meta, in_maps = _pack(**inputs)
    nc = _build_program(meta)
    res = run_bass_kernel_spmd(
        nc,
        in_maps,
        list(range(NCORES)),
        trace=trace,
        trace_cores=trace_cores,
    )
    n, nb = meta["n"], meta["nb"]
    out = np.empty((n, P), np.float32)
    for k in range(NCORES):
        out[k * nb : (k + 1) * nb] = np.asarray(res.results[k]["out"])[:nb]
    return out, res


def kernel(x, edge_index, weight, b):
    out, _ = _run(dict(x=x, edge_index=edge_index, weight=weight, b=b))
    return out


if __name__ == "__main__":
    rng = np.random.default_rng(0)
    n, e = 100000, 1600000
    x = rng.standard_normal((n, P), dtype=np.float32)
    ei = rng.integers(0, n, (2, e)).astype(np.int64)
    w = (rng.standard_normal((P, P)) / np.sqrt(P)).astype(np.float32)
    bb = (rng.standard_normal(P) * 0.02).astype(np.float32)
    out = kernel(x, ei, w, bb)
    print("out", out.shape, out.dtype)


# revision 7
# speedup vs baseline: 1.7058x; 1.1103x over previous
"""GCNConv (PyG-faithful, normalize=True, add_self_loops=True) on 8 Trainium2
NeuronCores via Bass/Tile.

Strategy (1D graph/data parallel), v3:
  - Nodes are partitioned across the 8 cores (12500 rows each, padded to
    12544 = 98 blocks of 128).
  - Phase A: each core computes h_k = (dinv*x)_k @ W in bf16 (dinv folded
    host-side), storing the shard in four row-quarters.  After each quarter
    a quarter-wise AllGather replicates it into that quarter's chunk of the
    global message table (quarter-major layout), pipelining the collective
    with compute and with phase-B gathers.
  - Phase B: each core owns 1/8 of the destination nodes.  Edges (incl.
    self-loops) are host-sorted by (dst window of 7 blocks, table chunk,
    dst block); per (window, chunk) section one dma_gather (SWDGE, queue=c)
    fetches g[src] rows (bf16).  Section tails are padded with index 0
    (harmless row) whose sel columns are zero.  One-hot sel tiles (fp8) are
    generated ON-CHIP per window by a single VectorE is_equal over a
    broadcast iota vs per-tile dl columns (dls input, bf16), eliminating
    the 35MB/core sel stream.  TensorE matmuls segment-sum each block's
    messages into PSUM; tiles straddling block boundaries feed both blocks'
    matmuls with separate sel columns.  Epilogue scales by dinv_dst on
    ScalarE, adds bias on VectorE, stores per window.
"""

import sys

if "/opt/trn_rl_repo" not in sys.path:
    sys.path.insert(0, "/opt/trn_rl_repo")

import numpy as np

P = 128          # partitions / tile edge count / feature dim
NCORES = 8
WBLK = 7         # dst blocks per window
NCHUNK = 4       # table chunks == phase-A quarters
MBUFS = 3        # msg tile buffers
SBUFS = 3        # sel window buffers
IBUFS = 3        # idx window buffers
OBUFS = 2


def _pack(x, edge_index, weight, b):
    """Host-side preprocessing: sharding, normalization, quarter-major table
    layout, per-window section packing, masked-dl columns for on-chip sel."""
    import ml_dtypes

    bf16 = ml_dtypes.bfloat16

    x = np.ascontiguousarray(np.asarray(x, dtype=np.float32))
    ei = np.asarray(edge_index)
    weight = np.ascontiguousarray(np.asarray(weight, dtype=np.float32))
    bias = np.asarray(b, dtype=np.float32).reshape(-1)

    n, nin = x.shape
    assert nin == P and weight.shape == (P, P)
    assert n % NCORES == 0
    nb = n // NCORES                      # nodes per core (12500)
    blocks = (nb + P - 1) // P            # blocks per core (98)
    nbp = blocks * P                      # padded nodes per core (12544)
    nw = blocks // WBLK                   # windows (14)
    assert blocks % WBLK == 0

    # quarters of the local shard (in blocks): 25,25,24,24
    qb = [blocks // NCHUNK + (1 if i < blocks % NCHUNK else 0) for i in range(NCHUNK)]
    qrows = [q * P for q in qb]
    qstart = np.concatenate([[0], np.cumsum(qrows)])
    chunk_rows = [NCORES * r for r in qrows]
    assert max(chunk_rows) < 32768

    src = ei[0].astype(np.int64)
    dst = ei[1].astype(np.int64)

    deg = np.bincount(dst, minlength=n).astype(np.float32) + 1.0
    dinv = 1.0 / np.sqrt(deg)

    loop = np.arange(n, dtype=np.int64)
    src_a = np.concatenate([src, loop])
    dst_a = np.concatenate([dst, loop])

    core = dst_a // nb
    dlc = dst_a - core * nb
    blk = dlc >> 7
    dl = (dlc & 127).astype(np.int16)
    win = blk // WBLK
    sub = (blk % WBLK).astype(np.int8)

    score = src_a // nb
    sloc = src_a % nb
    q = np.searchsorted(qstart[1:-1], sloc, side="right")
    rel = (score * np.asarray(qrows)[q] + (sloc - qstart[q])).astype(np.int16)

    key = (((core * nw + win) * NCHUNK + q) * WBLK + sub).astype(np.int64)
    order = np.argsort(key, kind="stable")
    karr, warr, qarr, subarr = core[order], win[order], q[order], sub[order]
    dlarr, relarr = dl[order], rel[order]

    nsec = nw * NCHUNK
    gk = (karr * nsec + warr * NCHUNK + qarr).astype(np.int64)
    gk2 = gk * WBLK + subarr
    cnt_sec = np.bincount(gk, minlength=NCORES * nsec).reshape(NCORES, nw, NCHUNK)
    cnt_sub = np.bincount(gk2, minlength=NCORES * nsec * WBLK).reshape(
        NCORES, nw, NCHUNK, WBLK
    )
    start_sub = np.cumsum(cnt_sub, axis=-1) - cnt_sub        # exclusive cumsum
    end_sub = start_sub + cnt_sub

    cap = cnt_sec.max(axis=0)                                # [nw, NCHUNK]
    t_s = -(-cap // P)

    tile_base = np.zeros((nw, NCHUNK), np.int64)
    wbase = np.zeros(nw + 1, np.int64)
    col = 0
    for w in range(nw):
        wbase[w] = col
        for c in range(NCHUNK):
            tile_base[w, c] = col
            col += int(t_s[w, c])
    s_tiles = int(col)
    wbase[nw] = col
    jmax = int((wbase[1:] - wbase[:-1]).max())

    # per-core packed idx / dl / sub arrays over the static layout
    gs = np.zeros(NCORES * nsec, np.int64)
    gs[1:] = np.cumsum(np.bincount(gk, minlength=NCORES * nsec))[:-1]
    rank = np.arange(gk.size, dtype=np.int64) - gs[gk]
    base_flat = (tile_base * P).reshape(-1)
    pos = base_flat[gk % nsec] + rank

    idx_lin = np.zeros((NCORES, s_tiles * P), np.int16)      # pads gather row 0
    dl_lin = np.full((NCORES, s_tiles * P), -1, np.int16)
    sub_lin = np.full((NCORES, s_tiles * P), -1, np.int8)
    idx_lin[karr, pos] = relarr
    dl_lin[karr, pos] = dlarr
    sub_lin[karr, pos] = subarr

    # static matmul tile ranges per (window, chunk, sub-block)
    anyb = (cnt_sub > 0).any(axis=0)                          # [nw, NCHUNK, WBLK]
    big = np.iinfo(np.int64).max
    lo_b = np.where(
        anyb, np.where(cnt_sub > 0, start_sub, big).min(axis=0) // P, 0
    )
    hi_b = np.where(
        anyb, -(-np.where(cnt_sub > 0, end_sub, 0).max(axis=0) // P), 0
    )

    # sel entries: (w, c, t, b) in program order; per-window ranges
    sel_list = []       # (w, c, t, b)
    wsel_base = np.zeros(nw + 1, np.int64)
    mm_meta = []        # mm_meta[w][b] = list of (c, t, scol)
    for w in range(nw):
        wsel_base[w] = len(sel_list)
        mm_w = [[] for _ in range(WBLK)]
        for c in range(NCHUNK):
            for bb in range(WBLK):
                for t in range(int(lo_b[w, c, bb]), int(hi_b[w, c, bb])):
                    scol = len(sel_list) - int(wsel_base[w])
                    mm_w[bb].append((c, t, scol))
                    sel_list.append((w, c, t, bb))
        mm_meta.append(mm_w)
    wsel_base[nw] = len(sel_list)
    s_sel = len(sel_list)
    selw_max = int((wsel_base[1:] - wsel_base[:-1]).max())
    sel_w = np.asarray([e[0] for e in sel_list], np.int64)
    sel_c = np.asarray([e[1] for e in sel_list], np.int64)
    sel_t = np.asarray([e[2] for e in sel_list], np.int64)
    sel_b = np.asarray([e[3] for e in sel_list], np.int16)

    # verify every real edge is covered by its block's static tile range
    t_of_pos = (pos - base_flat[gk % nsec]) // P
    lo_e = lo_b[warr, qarr, subarr]
    hi_e = hi_b[warr, qarr, subarr]
    assert (t_of_pos >= lo_e).all() and (t_of_pos < hi_e).all()

    # masked dl per sel entry: dls[e, sid] = dl if edge belongs to b else -1
    sel_gt = tile_base[sel_w, sel_c] + sel_t
    epos = sel_gt[:, None] * P + np.arange(P)[None, :]        # [s_sel, P]
    dls = np.empty((NCORES, P, s_sel), bf16)
    for k in range(NCORES):
        dle = dl_lin[k][epos]                                 # [s_sel, P]
        sbe = sub_lin[k][epos]
        m = (sbe == sel_b[:, None]) & (dle >= 0)
        dls[k] = np.where(m, dle, -1).astype(np.float32).T.astype(bf16)

    # wrap-16 + replicate to 128 partitions for dma_gather idx layout
    l16 = s_tiles * P // 16
    idx_wr = idx_lin.reshape(NCORES, l16, 16).transpose(0, 2, 1)
    idx_pack = np.ascontiguousarray(np.tile(idx_wr, (1, NCORES, 1)))

    iota = np.ascontiguousarray(
        np.tile(np.arange(P, dtype=np.float32)[None, :], (P, 1)).astype(bf16)
    )

    # per-core xT (dinv folded, bf16), dinv columns, bias
    xt = np.zeros((NCORES, P, nbp), bf16)
    dinv_t = np.zeros((NCORES, P, blocks), np.float32)
    for k in range(NCORES):
        xs = x[k * nb : (k + 1) * nb] * dinv[k * nb : (k + 1) * nb, None]
        xt[k, :, :nb] = xs.T.astype(bf16)
        dv = np.zeros(nbp, np.float32)
        dv[:nb] = dinv[k * nb : (k + 1) * nb]
        dinv_t[k] = dv.reshape(blocks, P).T
    w_bf = np.ascontiguousarray(weight.astype(bf16))
    bias_rep = np.ascontiguousarray(np.tile(bias[None, :], (P, 1)))

    meta = dict(
        n=n, nb=nb, blocks=blocks, nbp=nbp, nw=nw,
        qb=qb, qrows=qrows, chunk_rows=chunk_rows,
        t_s=t_s, tile_base=tile_base, wbase=wbase, s_tiles=s_tiles,
        jmax=jmax, l16=l16, s_sel=s_sel, selw_max=selw_max,
        wsel_base=wsel_base, mm_meta=mm_meta,
    )
    in_maps = [
        {
            "xt": xt[k],
            "w_in": w_bf,
            "bias": bias_rep,
            "dinv": dinv_t[k],
            "idxp": idx_pack[k],
            "dls": dls[k],
            "iota": iota,
        }
        for k in range(NCORES)
    ]
    return meta, in_maps


def _build_program(meta):
    from concourse import bass, bacc, mybir
    import concourse.tile as tile

    blocks = meta["blocks"]
    nbp = meta["nbp"]
    nw = meta["nw"]
    qb = meta["qb"]
    qrows = meta["qrows"]
    chunk_rows = meta["chunk_rows"]
    t_s = meta["t_s"]
    tile_base = meta["tile_base"]
    wbase = meta["wbase"]
    jmax = meta["jmax"]
    l16 = meta["l16"]
    s_sel = meta["s_sel"]
    selw_max = meta["selw_max"]
    wsel_base = meta["wsel_base"]
    mm_meta = meta["mm_meta"]

    f32 = mybir.dt.float32
    bf16 = mybir.dt.bfloat16
    fp8 = mybir.dt.float8e4
    i16 = mybir.dt.int16

    wl16 = [int(wbase[w + 1] - wbase[w]) * 8 for w in range(nw)]
    wl16_max = max(wl16)

    nc = bacc.Bacc(num_swdge_queues=4)
    xt_in = nc.declare_dram_parameter("xt", [P, nbp], bf16, isOutput=False)
    w_in = nc.declare_dram_parameter("w_in", [P, P], bf16, isOutput=False)
    bias_in = nc.declare_dram_parameter("bias", [P, P], f32, isOutput=False)
    dinv_in = nc.declare_dram_parameter("dinv", [P, blocks], f32, isOutput=False)
    idx_in = nc.declare_dram_parameter("idxp", [P, l16], i16, isOutput=False)
    dls_in = nc.declare_dram_parameter("dls", [P, s_sel], bf16, isOutput=False)
    iota_in = nc.declare_dram_parameter("iota", [P, P], bf16, isOutput=False)
    out_ext = nc.declare_dram_parameter("out", [nbp, P], f32, isOutput=True)

    h_q = [nc.dram_tensor(f"h_q{c}", [qrows[c], P], bf16) for c in range(NCHUNK)]
    g_t = [
        nc.dram_tensor(f"g_t{c}", [chunk_rows[c], P], bf16, addr_space="Shared")
        for c in range(NCHUNK)
    ]

    with tile.TileContext(nc) as tc:
        with tc.tile_pool(name="const", bufs=1) as cpool:
            w_sb = cpool.tile([P, P], bf16, tag="w")
            nc.sync.dma_start(out=w_sb[:], in_=w_in[:])
            bias_sb = cpool.tile([P, P], f32, tag="bias")
            nc.sync.dma_start(out=bias_sb[:], in_=bias_in[:])
            dinv_sb = cpool.tile([P, blocks], f32, tag="dinv")
            nc.sync.dma_start(out=dinv_sb[:], in_=dinv_in[:])
            iota_sb = cpool.tile([P, P], bf16, tag="iota")
            nc.sync.dma_start(out=iota_sb[:], in_=iota_in[:])
            dls_sb = cpool.tile([P, s_sel], bf16, tag="dls")
            nc.scalar.dma_start(out=dls_sb[:], in_=dls_in[:])

            # ---- phase A: h = (dinv*x) @ W per quarter + quarter AllGather
            with (
                tc.tile_pool(name="aph", bufs=2) as apool,
                tc.tile_pool(name="psA", bufs=2, space="PSUM") as psA,
            ):
                qs = 0
                for c in range(NCHUNK):
                    rows, qbt = qrows[c], qb[c]
                    xa = apool.tile([P, rows], bf16, tag="xa")
                    nc.sync.dma_start(out=xa[:], in_=xt_in[:, qs : qs + rows])
                    hq = apool.tile([P, qbt, P], bf16, tag="hq")
                    t = 0
                    while t < qbt:
                        g = min(4, qbt - t)
                        ph = psA.tile([P, 4, P], f32, tag="ph")
                        for j in range(g):
                            nc.tensor.matmul(
                                out=ph[:, j, :],
                                lhsT=xa[:, (t + j) * P : (t + j + 1) * P],
                                rhs=w_sb[:],
                                start=True,
                                stop=True,
                            )
                        nc.scalar.activation(
                            out=hq[:, t : t + g, :],
                            in_=ph[:, :g, :],
                            func=mybir.ActivationFunctionType.Copy,
                        )
                        t += g
                    nc.sync.dma_start(
                        out=h_q[c][:].rearrange("(t p) f -> p t f", p=P),
                        in_=hq[:],
                    )
                    nc.gpsimd.collective_compute(
                        "AllGather",
                        mybir.AluOpType.bypass,
                        replica_groups=[list(range(NCORES))],
                        ins=[h_q[c][:]],
                        outs=[g_t[c][:]],
                    )
                    qs += rows

            # ---- phase B: gather + on-chip sel + segment matmuls per window
            with (
                tc.tile_pool(name="msgp", bufs=MBUFS) as mpool,
                tc.tile_pool(name="selp", bufs=SBUFS) as spool,
                tc.tile_pool(name="idxp", bufs=IBUFS) as ipool,
                tc.tile_pool(name="outp", bufs=OBUFS) as opool,
                tc.tile_pool(name="psB", bufs=8, space="PSUM") as psB,
            ):
                for w in range(nw):
                    wb = int(wbase[w])
                    msg = mpool.tile([P, jmax, P], bf16, tag="msg")
                    idxw = ipool.tile([P, wl16_max], i16, tag="idxw")
                    nc.sync.dma_start(
                        out=idxw[:, : wl16[w]],
                        in_=idx_in[:, wb * 8 : wb * 8 + wl16[w]],
                    )
                    nsel = int(wsel_base[w + 1] - wsel_base[w])
                    ws0 = int(wsel_base[w])
                    selw = spool.tile([P, selw_max, P], fp8, tag="selw")
                    nc.vector.tensor_tensor(
                        out=selw[:, :nsel, :],
                        in0=iota_sb[:].unsqueeze(1).to_broadcast([P, nsel, P]),
                        in1=dls_sb[:, ws0 : ws0 + nsel]
                        .unsqueeze(2)
                        .to_broadcast([P, nsel, P]),
                        op=mybir.AluOpType.is_equal,
                    )
                    for c in range(NCHUNK):
                        tc_ = int(t_s[w, c])
                        if tc_ == 0:
                            continue
                        lo = int(tile_base[w, c]) - wb
                        nc.gpsimd.dma_gather(
                            out_ap=msg[:, lo : lo + tc_, :],
                            in_ap=g_t[c][:],
                            idxs_ap=idxw[:, lo * 8 : (lo + tc_) * 8],
                            num_idxs=tc_ * P,
                            num_idxs_reg=tc_ * P,
                            elem_size=P,
                            single_packet=False,
                            queue_num=c,
                        )
                    osb = opool.tile([P, WBLK, P], f32, tag="osb")
                    for bb in range(WBLK):
                        mml = mm_meta[w][bb]
                        assert mml
                        acc = psB.tile([P, P], f32, tag="acc")
                        for i, (c, t, scol) in enumerate(mml):
                            mcol = int(tile_base[w, c]) + t - wb
                            nc.tensor.matmul(
                                out=acc[:],
                                lhsT=selw[:, scol, :],
                                rhs=msg[:, mcol, :],
                                start=(i == 0),
                                stop=(i == len(mml) - 1),
                            )
                        gb = w * WBLK + bb
                        nc.scalar.activation(
                            out=osb[:, bb, :],
                            in_=acc[:],
                            func=mybir.ActivationFunctionType.Copy,
                            scale=dinv_sb[:, gb : gb + 1],
                        )
                    nc.vector.tensor_tensor(
                        out=osb[:],
                        in0=osb[:],
                        in1=bias_sb[:].unsqueeze(1).to_broadcast([P, WBLK, P]),
                        op=mybir.AluOpType.add,
                    )
                    nc.sync.dma_start(
                        out=out_ext[w * WBLK * P : (w + 1) * WBLK * P, :].rearrange(
                            "(j p) f -> p j f", p=P
                        ),
                        in_=osb[:],
                    )

    nc.finalize()
    return nc


def _run(inputs, trace=False, trace_cores=None):
    from concourse.bass_utils import run_bass_kernel_spmd

    meta, in_maps = _pack(**inputs)
    nc = _build_program(meta)
    res = run_bass_kernel_spmd(
        nc,
        in_maps,
        list(range(NCORES)),
        trace=trace,
        trace_cores=trace_cores,
    )
    n, nb = meta["n"], meta["nb"]
    out = np.empty((n, P), np.float32)
    for k in range(NCORES):
        out[k * nb : (k + 1) * nb] = np.asarray(res.results[k]["out"])[:nb]
    return out, res


def kernel(x, edge_index, weight, b):
    out, _ = _run(dict(x=x, edge_index=edge_index, weight=weight, b=b))
    return out


if __name__ == "__main__":
    rng = np.random.default_rng(0)
    n, e = 100000, 1600000
    x = rng.standard_normal((n, P), dtype=np.float32)
    ei = rng.integers(0, n, (2, e)).astype(np.int64)
    w = (rng.standard_normal((P, P)) / np.sqrt(P)).astype(np.float32)
    bb = (rng.standard_normal(P) * 0.02).astype(np.float32)
    out = kernel(x, ei, w, bb)
    print("out", out.shape, out.dtype)


# revision 18
# speedup vs baseline: 1.7290x; 1.0136x over previous
"""GCNConv (PyG-faithful, normalize=True, add_self_loops=True) on 8 Trainium2
NeuronCores via Bass/Tile.

Strategy (1D graph/data parallel), v3:
  - Nodes are partitioned across the 8 cores (12500 rows each, padded to
    12544 = 98 blocks of 128).
  - Phase A: each core computes h_k = (dinv*x)_k @ W in bf16 (dinv folded
    host-side), storing the shard in four row-quarters.  After each quarter
    a quarter-wise AllGather replicates it into that quarter's chunk of the
    global message table (quarter-major layout), pipelining the collective
    with compute and with phase-B gathers.
  - Phase B: each core owns 1/8 of the destination nodes.  Edges (incl.
    self-loops) are host-sorted by (dst window of 7 blocks, table chunk,
    dst block); per (window, chunk) section one dma_gather (SWDGE, queue=c)
    fetches g[src] rows (bf16).  Section tails are padded with index 0
    (harmless row) whose sel columns are zero.  One-hot sel tiles (fp8) are
    generated ON-CHIP per window by a single VectorE is_equal over a
    broadcast iota vs per-tile dl columns (dls input, bf16), eliminating
    the 35MB/core sel stream.  TensorE matmuls segment-sum each block's
    messages into PSUM; tiles straddling block boundaries feed both blocks'
    matmuls with separate sel columns.  Epilogue scales by dinv_dst on
    ScalarE, adds bias on VectorE, stores per window.
"""

import sys

if "/opt/trn_rl_repo" not in sys.path:
    sys.path.insert(0, "/opt/trn_rl_repo")

import numpy as np

P = 128          # partitions / tile edge count / feature dim
NCORES = 8
WBLK = 7         # dst blocks per window
NCHUNK = 4       # table chunks == phase-A quarters
MBUFS = 13       # per-section msg tile buffers (~3 windows of lookahead)
SBUFS = 2        # sel window buffers
IBUFS = 3        # idx window buffers
OBUFS = 2


def _pack(x, edge_index, weight, b):
    """Host-side preprocessing: sharding, normalization, quarter-major table
    layout, per-window section packing, masked-dl columns for on-chip sel."""
    import ml_dtypes

    bf16 = ml_dtypes.bfloat16

    x = np.ascontiguousarray(np.asarray(x, dtype=np.float32))
    ei = np.asarray(edge_index)
    weight = np.ascontiguousarray(np.asarray(weight, dtype=np.float32))
    bias = np.asarray(b, dtype=np.float32).reshape(-1)

    n, nin = x.shape
    assert nin == P and weight.shape == (P, P)
    assert n % NCORES == 0
    nb = n // NCORES                      # nodes per core (12500)
    blocks = (nb + P - 1) // P            # blocks per core (98)
    nbp = blocks * P                      # padded nodes per core (12544)
    nw = blocks // WBLK                   # windows (14)
    assert blocks % WBLK == 0

    # quarters of the local shard (in blocks): 25,25,24,24
    qb = [blocks // NCHUNK + (1 if i < blocks % NCHUNK else 0) for i in range(NCHUNK)]
    qrows = [q * P for q in qb]
    qstart = np.concatenate([[0], np.cumsum(qrows)])
    chunk_rows = [NCORES * r for r in qrows]
    assert max(chunk_rows) < 32768

    src = ei[0].astype(np.int64)
    dst = ei[1].astype(np.int64)

    deg = np.bincount(dst, minlength=n).astype(np.float32) + 1.0
    dinv = 1.0 / np.sqrt(deg)

    loop = np.arange(n, dtype=np.int64)
    src_a = np.concatenate([src, loop])
    dst_a = np.concatenate([dst, loop])

    core = dst_a // nb
    dlc = dst_a - core * nb
    blk = dlc >> 7
    dl = (dlc & 127).astype(np.int16)
    win = blk // WBLK
    sub = (blk % WBLK).astype(np.int8)

    score = src_a // nb
    sloc = src_a % nb
    q = np.searchsorted(qstart[1:-1], sloc, side="right")
    rel = (score * np.asarray(qrows)[q] + (sloc - qstart[q])).astype(np.int16)

    key = (((core * nw + win) * NCHUNK + q) * WBLK + sub).astype(np.int64)
    order = np.argsort(key, kind="stable")
    karr, warr, qarr, subarr = core[order], win[order], q[order], sub[order]
    dlarr, relarr = dl[order], rel[order]

    nsec = nw * NCHUNK
    gk = (karr * nsec + warr * NCHUNK + qarr).astype(np.int64)
    gk2 = gk * WBLK + subarr
    cnt_sec = np.bincount(gk, minlength=NCORES * nsec).reshape(NCORES, nw, NCHUNK)
    cnt_sub = np.bincount(gk2, minlength=NCORES * nsec * WBLK).reshape(
        NCORES, nw, NCHUNK, WBLK
    )
    start_sub = np.cumsum(cnt_sub, axis=-1) - cnt_sub        # exclusive cumsum
    end_sub = start_sub + cnt_sub

    cap = cnt_sec.max(axis=0)                                # [nw, NCHUNK]
    t_s = -(-cap // P)

    tile_base = np.zeros((nw, NCHUNK), np.int64)
    wbase = np.zeros(nw + 1, np.int64)
    col = 0
    for w in range(nw):
        wbase[w] = col
        for c in range(NCHUNK):
            tile_base[w, c] = col
            col += int(t_s[w, c])
    s_tiles = int(col)
    wbase[nw] = col
    jmax = int((wbase[1:] - wbase[:-1]).max())

    # per-core packed idx / dl / sub arrays over the static layout
    gs = np.zeros(NCORES * nsec, np.int64)
    gs[1:] = np.cumsum(np.bincount(gk, minlength=NCORES * nsec))[:-1]
    rank = np.arange(gk.size, dtype=np.int64) - gs[gk]
    base_flat = (tile_base * P).reshape(-1)
    pos = base_flat[gk % nsec] + rank

    idx_lin = np.zeros((NCORES, s_tiles * P), np.int16)      # pads gather row 0
    dl_lin = np.full((NCORES, s_tiles * P), -1, np.int16)
    sub_lin = np.full((NCORES, s_tiles * P), -1, np.int8)
    idx_lin[karr, pos] = relarr
    dl_lin[karr, pos] = dlarr
    sub_lin[karr, pos] = subarr

    # static matmul tile ranges per (window, chunk, sub-block)
    anyb = (cnt_sub > 0).any(axis=0)                          # [nw, NCHUNK, WBLK]
    big = np.iinfo(np.int64).max
    lo_b = np.where(
        anyb, np.where(cnt_sub > 0, start_sub, big).min(axis=0) // P, 0
    )
    hi_b = np.where(
        anyb, -(-np.where(cnt_sub > 0, end_sub, 0).max(axis=0) // P), 0
    )

    # sel entries: (w, c, t, b) in program order; per-window ranges
    sel_list = []       # (w, c, t, b)
    wsel_base = np.zeros(nw + 1, np.int64)
    mm_meta = []        # mm_meta[w][c][b] = list of (t, scol)
    for w in range(nw):
        wsel_base[w] = len(sel_list)
        mm_w = [[[] for _ in range(WBLK)] for _ in range(NCHUNK)]
        for c in range(NCHUNK):
            for bb in range(WBLK):
                for t in range(int(lo_b[w, c, bb]), int(hi_b[w, c, bb])):
                    scol = len(sel_list) - int(wsel_base[w])
                    mm_w[c][bb].append((t, scol))
                    sel_list.append((w, c, t, bb))
        mm_meta.append(mm_w)
    wsel_base[nw] = len(sel_list)
    s_sel = len(sel_list)
    selw_max = int((wsel_base[1:] - wsel_base[:-1]).max())
    sel_w = np.asarray([e[0] for e in sel_list], np.int64)
    sel_c = np.asarray([e[1] for e in sel_list], np.int64)
    sel_t = np.asarray([e[2] for e in sel_list], np.int64)
    sel_b = np.asarray([e[3] for e in sel_list], np.int16)

    # verify every real edge is covered by its block's static tile range
    t_of_pos = (pos - base_flat[gk % nsec]) // P
    lo_e = lo_b[warr, qarr, subarr]
    hi_e = hi_b[warr, qarr, subarr]
    assert (t_of_pos >= lo_e).all() and (t_of_pos < hi_e).all()

    # masked dl per sel entry: dls[e, sid] = dl if edge belongs to b else -1
    sel_gt = tile_base[sel_w, sel_c] + sel_t
    epos = sel_gt[:, None] * P + np.arange(P)[None, :]        # [s_sel, P]
    dls = np.empty((NCORES, P, s_sel), np.int8)
    for k in range(NCORES):
        dle = dl_lin[k][epos]                                 # [s_sel, P]
        sbe = sub_lin[k][epos]
        m = (sbe == sel_b[:, None]) & (dle >= 0)
        dls[k] = np.where(m, dle, -1).astype(np.int8).T

    # wrap-16 + replicate to 128 partitions for dma_gather idx layout
    l16 = s_tiles * P // 16
    idx_wr = idx_lin.reshape(NCORES, l16, 16).transpose(0, 2, 1)
    idx_pack = np.ascontiguousarray(np.tile(idx_wr, (1, NCORES, 1)))

    iota = np.ascontiguousarray(
        np.tile(np.arange(P, dtype=np.int8)[None, :], (P, 1))
    )

    # per-core xT (dinv folded, bf16), dinv columns, bias
    xt = np.zeros((NCORES, P, nbp), bf16)
    dinv_t = np.zeros((NCORES, P, blocks), np.float32)
    for k in range(NCORES):
        xs = x[k * nb : (k + 1) * nb] * dinv[k * nb : (k + 1) * nb, None]
        xt[k, :, :nb] = xs.T.astype(bf16)
        dv = np.zeros(nbp, np.float32)
        dv[:nb] = dinv[k * nb : (k + 1) * nb]
        dinv_t[k] = dv.reshape(blocks, P).T
    w_bf = np.ascontiguousarray(weight.astype(bf16))
    bias_rep = np.ascontiguousarray(np.tile(bias[None, :], (P, 1)))

    meta = dict(
        n=n, nb=nb, blocks=blocks, nbp=nbp, nw=nw,
        qb=qb, qrows=qrows, chunk_rows=chunk_rows,
        t_s=t_s, tile_base=tile_base, wbase=wbase, s_tiles=s_tiles,
        jmax=jmax, l16=l16, s_sel=s_sel, selw_max=selw_max,
        wsel_base=wsel_base, mm_meta=mm_meta, tmax_sec=int(t_s.max()),
    )
    in_maps = [
        {
            "xt": xt[k],
            "w_in": w_bf,
            "bias": bias_rep,
            "dinv": dinv_t[k],
            "idxp": idx_pack[k],
            "dls": dls[k],
            "iota": iota,
        }
        for k in range(NCORES)
    ]
    return meta, in_maps


def _build_program(meta):
    from concourse import bass, bacc, mybir
    import concourse.tile as tile

    blocks = meta["blocks"]
    nbp = meta["nbp"]
    nw = meta["nw"]
    qb = meta["qb"]
    qrows = meta["qrows"]
    chunk_rows = meta["chunk_rows"]
    t_s = meta["t_s"]
    tile_base = meta["tile_base"]
    wbase = meta["wbase"]
    jmax = meta["jmax"]
    l16 = meta["l16"]
    s_sel = meta["s_sel"]
    selw_max = meta["selw_max"]
    wsel_base = meta["wsel_base"]
    mm_meta = meta["mm_meta"]

    f32 = mybir.dt.float32
    bf16 = mybir.dt.bfloat16
    fp8 = mybir.dt.float8e4
    i16 = mybir.dt.int16
    i8 = mybir.dt.int8

    wl16 = [int(wbase[w + 1] - wbase[w]) * 8 for w in range(nw)]
    wl16_max = max(wl16)
    tmax_sec = meta["tmax_sec"]

    nc = bacc.Bacc(num_swdge_queues=4)
    xt_in = nc.declare_dram_parameter("xt", [P, nbp], bf16, isOutput=False)
    w_in = nc.declare_dram_parameter("w_in", [P, P], bf16, isOutput=False)
    bias_in = nc.declare_dram_parameter("bias", [P, P], f32, isOutput=False)
    dinv_in = nc.declare_dram_parameter("dinv", [P, blocks], f32, isOutput=False)
    idx_in = nc.declare_dram_parameter("idxp", [P, l16], i16, isOutput=False)
    dls_in = nc.declare_dram_parameter("dls", [P, s_sel], i8, isOutput=False)
    iota_in = nc.declare_dram_parameter("iota", [P, P], i8, isOutput=False)
    out_ext = nc.declare_dram_parameter("out", [nbp, P], f32, isOutput=True)

    h_q = [nc.dram_tensor(f"h_q{c}", [qrows[c], P], bf16) for c in range(NCHUNK)]
    g_t = [
        nc.dram_tensor(f"g_t{c}", [chunk_rows[c], P], bf16, addr_space="Shared")
        for c in range(NCHUNK)
    ]
    warm_in = nc.dram_tensor("warm_in", [1, P], bf16)
    warm_out = nc.dram_tensor("warm_out", [NCORES, P], bf16, addr_space="Shared")

    with tile.TileContext(nc) as tc:
        # tiny warmup collective to absorb the ncfw first-collective setup
        nc.gpsimd.collective_compute(
            "AllGather",
            mybir.AluOpType.bypass,
            replica_groups=[list(range(NCORES))],
            ins=[warm_in[:]],
            outs=[warm_out[:]],
        )
        with tc.tile_pool(name="const", bufs=1) as cpool:
            w_sb = cpool.tile([P, P], bf16, tag="w")
            nc.sync.dma_start(out=w_sb[:], in_=w_in[:])
            bias_sb = cpool.tile([P, P], f32, tag="bias")
            nc.sync.dma_start(out=bias_sb[:], in_=bias_in[:])
            dinv_sb = cpool.tile([P, blocks], f32, tag="dinv")
            nc.sync.dma_start(out=dinv_sb[:], in_=dinv_in[:])
            iota_sb = cpool.tile([P, P], i8, tag="iota")
            nc.sync.dma_start(out=iota_sb[:], in_=iota_in[:])
            dls_sb = cpool.tile([P, s_sel], i8, tag="dls")
            nc.scalar.dma_start(out=dls_sb[:], in_=dls_in[:])

            # ---- phase A: h = (dinv*x) @ W per quarter + quarter AllGather
            with (
                tc.tile_pool(name="aph", bufs=2) as apool,
                tc.tile_pool(name="psA", bufs=2, space="PSUM") as psA,
            ):
                qs = 0
                for c in range(NCHUNK):
                    rows, qbt = qrows[c], qb[c]
                    xa = apool.tile([P, rows], bf16, tag="xa")
                    nc.sync.dma_start(out=xa[:], in_=xt_in[:, qs : qs + rows])
                    hq = apool.tile([P, qbt, P], bf16, tag="hq")
                    t = 0
                    while t < qbt:
                        g = min(4, qbt - t)
                        ph = psA.tile([P, 4, P], f32, tag="ph")
                        for j in range(g):
                            nc.tensor.matmul(
                                out=ph[:, j, :],
                                lhsT=xa[:, (t + j) * P : (t + j + 1) * P],
                                rhs=w_sb[:],
                                start=True,
                                stop=True,
                            )
                        nc.scalar.activation(
                            out=hq[:, t : t + g, :],
                            in_=ph[:, :g, :],
                            func=mybir.ActivationFunctionType.Copy,
                        )
                        t += g
                    nc.sync.dma_start(
                        out=h_q[c][:].rearrange("(t p) f -> p t f", p=P),
                        in_=hq[:],
                    )
                    nc.gpsimd.collective_compute(
                        "AllGather",
                        mybir.AluOpType.bypass,
                        replica_groups=[list(range(NCORES))],
                        ins=[h_q[c][:]],
                        outs=[g_t[c][:]],
                    )
                    qs += rows

            # ---- phase B: gather + on-chip sel + segment matmuls per window
            with (
                tc.tile_pool(name="msgp", bufs=MBUFS) as mpool,
                tc.tile_pool(name="selp", bufs=SBUFS) as spool,
                tc.tile_pool(name="idxp", bufs=IBUFS) as ipool,
                tc.tile_pool(name="outp", bufs=OBUFS) as opool,
                tc.tile_pool(name="psB", bufs=8, space="PSUM") as psB,
            ):
                for w in range(nw):
                    wb = int(wbase[w])
                    idxw = ipool.tile([P, wl16_max], i16, tag="idxw")
                    nc.scalar.dma_start(
                        out=idxw[:, : wl16[w]],
                        in_=idx_in[:, wb * 8 : wb * 8 + wl16[w]],
                    )
                    nsel = int(wsel_base[w + 1] - wsel_base[w])
                    ws0 = int(wsel_base[w])
                    selw = spool.tile([P, selw_max, P], fp8, tag="selw")
                    nc.vector.tensor_tensor(
                        out=selw[:, :nsel, :],
                        in0=iota_sb[:].unsqueeze(1).to_broadcast([P, nsel, P]),
                        in1=dls_sb[:, ws0 : ws0 + nsel]
                        .unsqueeze(2)
                        .to_broadcast([P, nsel, P]),
                        op=mybir.AluOpType.is_equal,
                    )
                    msgs = [None] * NCHUNK
                    for c in range(NCHUNK):
                        tc_ = int(t_s[w, c])
                        if tc_ == 0:
                            continue
                        lo = int(tile_base[w, c]) - wb
                        mt = mpool.tile(
                            [P, tmax_sec, P], bf16, tag="msg", name=f"msg_{w}_{c}"
                        )
                        msgs[c] = mt
                        nc.gpsimd.dma_gather(
                            out_ap=mt[:, :tc_, :],
                            in_ap=g_t[c][:],
                            idxs_ap=idxw[:, lo * 8 : (lo + tc_) * 8],
                            num_idxs=tc_ * P,
                            num_idxs_reg=tc_ * P,
                            elem_size=P,
                            single_packet=False,
                            queue_num=c,
                        )
                    # chunk-outer matmul passes: section c's matmuls run as
                    # soon as its gather lands, releasing the msg tile early
                    accs = [
                        psB.tile([P, P], f32, tag="acc", name=f"acc_{w}_{bb}")
                        for bb in range(WBLK)
                    ]
                    tot = [
                        sum(len(mm_meta[w][c][bb]) for c in range(NCHUNK))
                        for bb in range(WBLK)
                    ]
                    done = [0] * WBLK
                    for c in range(NCHUNK):
                        for bb in range(WBLK):
                            for (t, scol) in mm_meta[w][c][bb]:
                                nc.tensor.matmul(
                                    out=accs[bb][:],
                                    lhsT=selw[:, scol, :],
                                    rhs=msgs[c][:, t, :],
                                    start=(done[bb] == 0),
                                    stop=(done[bb] == tot[bb] - 1),
                                )
                                done[bb] += 1
                    osb = opool.tile([P, WBLK, P], f32, tag="osb")
                    for bb in range(WBLK):
                        assert done[bb] == tot[bb] and tot[bb] > 0
                        gb = w * WBLK + bb
                        nc.scalar.activation(
                            out=osb[:, bb, :],
                            in_=accs[bb][:],
                            func=mybir.ActivationFunctionType.Copy,
                            scale=dinv_sb[:, gb : gb + 1],
                        )
                    nc.vector.tensor_tensor(
                        out=osb[:],
                        in0=osb[:],
                        in1=bias_sb[:].unsqueeze(1).to_broadcast([P, WBLK, P]),
                        op=mybir.AluOpType.add,
                    )
                    nc.sync.dma_start(
                        out=out_ext[w * WBLK * P : (w + 1) * WBLK * P, :].rearrange(
                            "(j p) f -> p j f", p=P
                        ),
                        in_=osb[:],
                    )

    nc.finalize()
    return nc


def _run(inputs, trace=False, trace_cores=None):
    from concourse.bass_utils import run_bass_kernel_spmd

    meta, in_maps = _pack(**inputs)
    nc = _build_program(meta)
    res = run_bass_kernel_spmd(
        nc,
        in_maps,
        list(range(NCORES)),
        trace=trace,
        trace_cores=trace_cores,
    )
    n, nb = meta["n"], meta["nb"]
    out = np.empty((n, P), np.float32)
    for k in range(NCORES):
        out[k * nb : (k + 1) * nb] = np.asarray(res.results[k]["out"])[:nb]
    return out, res


def kernel(x, edge_index, weight, b):
    out, _ = _run(dict(x=x, edge_index=edge_index, weight=weight, b=b))
    return out


if __name__ == "__main__":
    rng = np.random.default_rng(0)
    n, e = 100000, 1600000
    x = rng.standard_normal((n, P), dtype=np.float32)
    ei = rng.integers(0, n, (2, e)).astype(np.int64)
    w = (rng.standard_normal((P, P)) / np.sqrt(P)).astype(np.float32)
    bb = (rng.standard_normal(P) * 0.02).astype(np.float32)
    out = kernel(x, ei, w, bb)
    print("out", out.shape, out.dtype)
